# revision 22
# baseline (speedup 1.0000x reference)
"""DFSMN (order-9 IIR + 2-tap lookahead FIR along frames) on 8 Trainium2 cores.

Math: per (b, h, d) sequence the reference is an LTI filter along frames,
out = g_d * v (combined impulse response g, lags -2..inf, |g| below noise
past lag ~61).  The output is dominated by the identity tap (g[0] ~= 1,
all other taps ~0.05), so the kernel computes only the CORRECTION

    c = (G - I) v        (RMS(c) ~= 0.17 * RMS(out))

and the host adds back the exact fp32 v it already holds.  Every fp8
quantization error (x, w, y) then scales with RMS(c), not RMS(out):
end-to-end rel err ~= 8e-3 vs the 2e-2 gate, at HALF the bf16 traffic.

Blocking: 64-frame output blocks; block b needs lags -2..61 ->
windows b and b-1 of the (+2-shifted) input:

    c_blk[b] = Wcur_d^T @ win[b] + Wprev_d^T @ win[b-1]

with Wcur/Wprev [64x64] Toeplitz slices of g' = g - delta, scaled by 16
so the decaying tail clears the fp8 subnormal floor (host divides by 16).

Packing: 2 channels per 128 partitions (chA rows 0:64, chB rows 64:128),
PE quadrant tiling at (0,0)/(64,64) -- odd pairs cross to (0,64)/(64,0)
so consecutive pairs touch disjoint array quadrants -> both channels'
matmuls run concurrently on the 128x128 array; windows x 64 (b,h) seqs
ride the free dim (16*64 = 1024 cols per channel-pair).

Per-core traffic (64 channels = 32 pairs): x 4.19MB + w 0.52MB +
y 4.19MB = 8.9MB fp8 -> ~25us HBM floor at 358 GB/s.

Window -1 of block 0 holds v[0:2] (rank-2): applied on the host in f64
together with nothing else -- the (G-I) form needs no other boundary fix.

Per-core tensors:
    x  [128, 32*1024] f8e4   col = q*1024 + win*64 + bh; part j<64 chA
                             frame j of win, j>=64 chB (ch = 2q, 2q+1)
    w  [128, 32*128]  f8e4   col = q*128 + {0:64 cur | 64:128 prev} lhsT
    y  [128, 32*1024] f8e4   col = q*1024 + blk*64 + bh; part i<64 chA
                             frame 64*blk+i, i>=64 chB; value = 16*c
"""

import numpy as np

import concourse.bass as bass
import concourse.bacc as bacc
import concourse.mybir as mybir
from concourse import tile
from concourse import bass_utils

B, H, T, D = 16, 4, 1024, 512
N_CORES = 8
DC = D // N_CORES          # 64 channels per core
PAIRS = DC // 2            # 32 channel pairs per core
BH = B * H                 # 64 sequences (matmul free dim)
NWIN = 16                  # 64-frame windows
FREE = NWIN * BH           # 1024 matmul free dim per pair
WCOL = 128                 # w cols per pair: cur [64] + prev [64]
LMAX = 61                  # last kept lag of the impulse response
WSCALE = 16.0              # host-side weight scale (psum = 16*c)
F32 = mybir.dt.float32
BF16 = mybir.dt.bfloat16
F8 = mybir.dt.float8e4

_NC_CACHE: dict = {}


def _build_nc(pairs: int = PAIRS):
    nc = bacc.Bacc("TRN2", target_bir_lowering=False, debug=False)
    x = nc.dram_tensor("x", [128, pairs * FREE], F8, kind="ExternalInput")
    w = nc.dram_tensor("w", [128, pairs * WCOL], F8, kind="ExternalInput")
    y = nc.dram_tensor("y", [128, pairs * FREE], F8, kind="ExternalOutput")
    xap, wap, yap = x.ap(), w.ap(), y.ap()
    # y-store groups (pairs): big in the middle, small last so the final
    # store's transfer+receipt tail is short
    YGROUPS = [(0, 8), (8, 8), (16, 8), (24, 4), (28, 2), (30, 2)]
    ymap = {}
    for s, n in YGROUPS:
        ymap[s + n - 1] = (s, n)       # trigger store on the group's last pair
    ystart = {s: (s, n) for s, n in YGROUPS}
    # ramped x load groups (pairs): small first tiles arrive early so the
    # matmul stream starts right after the PE warm-up burst
    GROUPS = [(0, 1), (1, 1), (2, 2), (4, 8), (12, 10), (22, 10)]
    gmap = {}                          # q -> (group_start, group_size)
    for s, n in GROUPS:
        for q in range(s, s + n):
            gmap[q] = (s, n)
    WGROUPS = [(0, 4), (4, 28)]

    with tile.TileContext(nc) as tc:
        with tc.tile_pool(name="xp", bufs=6) as xp, \
             tc.tile_pool(name="wp", bufs=len(WGROUPS)) as wp, \
             tc.tile_pool(name="op", bufs=3) as op, \
             tc.tile_pool(name="dp", bufs=1) as dp, \
             tc.tile_pool(name="pp", bufs=3, space="PSUM") as pp, \
             tc.tile_pool(name="pwp", bufs=1, space="PSUM") as pwp:
            # PE warm-up: HAM clock gate keeps PE at 1.2 GHz until ~3.4us of
            # sustained activity; burn the DMA-head time on dummy matmuls.
            # memset on DVE -- its preamble finishes ~3us before GpSimd's,
            # so the warm-up burst starts right after the framework barrier
            # instead of idling behind the first x DMA.
            # memset on DVE -- its preamble finishes ~2.5us before GpSimd's,
            # so the warm-up burst starts right after the framework barrier
            dummy = dp.tile([128, 512], BF16, name="dummy")
            nc.vector.memset(dummy, 0.0)
            pw = pwp.tile([128, 512], F32, name="pw")
            for _ in range(4):
                nc.tensor.matmul(pw, lhsT=dummy[:, 0:128], rhs=dummy,
                                 start=True, stop=True)
            # w (0.52MB) loads early on the scalar HWDGE queue, stays resident
            wtiles = {}
            for s, n in WGROUPS:
                wt = wp.tile([128, n * WCOL], F8, name="wt")
                wsrc = wap.copy()
                wsrc.ap = wsrc.ap[:0] + [[pairs * WCOL, 128], [1, n * WCOL]]
                wsrc.offset = s * WCOL
                nc.scalar.dma_start(out=wt, in_=wsrc)
                wtiles[s] = wt
            wmap = {}
            for s, n in WGROUPS:
                for q in range(s, s + n):
                    wmap[q] = (s, n)
            xt = yt = None
            xbase = None
            for q in range(pairs):
                if gmap[q][0] == q:
                    s, n = gmap[q]
                    xt = xp.tile([128, n * FREE], F8, name="xt")
                    src = xap.copy()
                    src.ap = src.ap[:0] + [[pairs * FREE, 128], [1, n * FREE]]
                    src.offset = s * FREE
                    nc.sync.dma_start(out=xt, in_=src)
                    xbase = s
                if q in ystart:
                    ys, yn = ystart[q]
                    yt = op.tile([128, yn * FREE], F8, name="yt")
                xv = xt[:, (q - xbase) * FREE:(q - xbase + 1) * FREE]
                ws, _ = wmap[q]
                wt = wtiles[ws]
                wo = (q - ws) * WCOL
                ps = pp.tile([128, FREE], F32, name="ps")
                # quadrant-tiled matmuls: chA on (0,0), chB on (64,64) run
                # concurrently; cur covers all 16 blocks, prev covers blocks
                # 1..15 via the 64-col shift.  Block 0's prev-window term is
                # rank-2 in v[0:2] and applied on the host.  Cols 0:64 never
                # see a stop=True -> skip the sim's accumulation-group check.
                # Each matmul's PSUM output must fit one 2KB bank -> split
                # the 1024-col free dim into 512-col halves.  Issue order
                # strictly alternates the (0,0) / (64,64) PE quadrants so
                # every LDWEIGHTS targets a row group that differs from the
                # in-flight matmul's and gets pulled ahead (hidden).
                HB = FREE // 2
                # odd pairs swap their PSUM row halves (PE tiles (0,64) and
                # (64,0) instead of (0,0)/(64,64)) so consecutive pairs touch
                # disjoint array quadrants; the host unswaps when unpacking.
                swap = 64 if (q % 2) else 0
                for c0, c1, wd, st in (
                        (0, HB, 0, True),          # cur, half 0
                        (HB, FREE, 0, True),       # cur, half 1
                        (64, HB, 64, False),       # prev, half 0
                        (HB, FREE, 64, False)):    # prev, half 1
                    sh = 0 if st else 64
                    for base in (0, 64):
                        ob = base ^ swap
                        nc.tensor.matmul(
                            ps[ob:ob + 64, c0:c1],
                            lhsT=wt[base:base + 64, wo + wd:wo + wd + 64],
                            rhs=xv[base:base + 64, c0 - sh:c1 - sh],
                            start=st, stop=not st, skip_group_check=True)
                # alternate PSUM evacuation between DVE and ACT (GpSimd has
                # no PSUM port)
                ysl = yt[:, (q - ys) * FREE:(q - ys + 1) * FREE]
                if q % 2 == 0:
                    nc.vector.tensor_copy(ysl, ps)
                else:
                    nc.scalar.copy(ysl, ps)
                if q in ymap:
                    gs, gn = ymap[q]
                    dst = yap.copy()
                    dst.ap = dst.ap[:0] + [[pairs * FREE, 128], [1, gn * FREE]]
                    dst.offset = gs * FREE
                    # ACT's HWDGE ring (separate from Sync's, which carries
                    # the x loads): faster issue than SWDGE and no slow
                    # gpsimd descriptor-ring drain in the postamble
                    nc.scalar.dma_start(out=dst, in_=yt)
                    # filler matmul bridges DMA-jitter stalls so the HAM
                    # clock gate never re-throttles mid-stream
                    if q < pairs - 1:
                        nc.tensor.matmul(pw, lhsT=dummy[:, 0:128],
                                         rhs=dummy, start=True, stop=True)
    nc.compile()
    return nc


def _get_nc(pairs: int = PAIRS):
    if pairs not in _NC_CACHE:
        _NC_CACHE[pairs] = _build_nc(pairs)
    return _NC_CACHE[pairs]


def _build_filters(l_filter: np.ndarray, r_filter: np.ndarray):
    """Returns Wcur, Wprev [64, 64, D] float64 lhsT Toeplitz blocks of the
    correction filter g' = g - delta (unscaled), truncated past lag LMAX."""
    c = l_filter[1:].astype(np.float64)            # (9, D) IIR coeffs
    d = c.shape[1]
    n_a = LMAX + 3
    a = np.zeros((n_a, d))
    a[0] = 1.0
    for n in range(1, n_a):
        for k in range(1, min(9, n) + 1):
            a[n] += c[k - 1] * a[n - k]
    q0 = 1.0 + l_filter[0].astype(np.float64)
    q1 = r_filter[0].astype(np.float64)
    q2 = r_filter[1].astype(np.float64)

    # gp[i] = correction tap at lag i-2, i in [0, LMAX+2]
    gp = np.zeros((LMAX + 3, d))
    gp[0] = q2 * a[0]
    gp[1] = q1 * a[0] + q2 * a[1]
    for lag in range(0, LMAX + 1):
        gp[lag + 2] = q0 * a[lag] + q1 * a[lag + 1] + q2 * a[lag + 2]
    gp[2] -= 1.0                                   # subtract identity

    jj = np.arange(64)[:, None]
    ii = np.arange(64)[None, :]
    lag_cur = ii - jj - 2                          # [-65, 61]
    lag_prev = ii - jj + 62                        # [-1, 125]
    Wcur = np.zeros((64, 64, d))
    Wprev = np.zeros((64, 64, d))
    mc = (lag_cur >= -2) & (lag_cur <= LMAX)
    mp = (lag_prev >= -2) & (lag_prev <= LMAX)
    Wcur[mc] = gp[(lag_cur + 2)[mc]]
    Wprev[mp] = gp[(lag_prev + 2)[mp]]
    return Wcur, Wprev


def _make_in_maps(v, l_filter, r_filter, n_cores=N_CORES):
    import ml_dtypes
    f8 = ml_dtypes.float8_e4m3
    Wcur, Wprev = _build_filters(l_filter, r_filter)
    # w lhsT layout: [part (half, j), pairall, col (cur|prev, i)]
    wsc = (Wcur * WSCALE).astype(np.float32)       # (64j, 64i, D)
    wsp = (Wprev * WSCALE).astype(np.float32)
    wall = np.empty((2, 64, D // 2, 2, 64), np.float32)
    # ch = 2*qall + half ; wall[half, j, qall, 0, i] = Wcur[j, i, ch]
    wall[0, :, :, 0, :] = wsc[:, :, 0::2].transpose(0, 2, 1)
    wall[0, :, :, 1, :] = wsp[:, :, 0::2].transpose(0, 2, 1)
    wall[1, :, :, 0, :] = wsc[:, :, 1::2].transpose(0, 2, 1)
    wall[1, :, :, 1, :] = wsp[:, :, 1::2].transpose(0, 2, 1)
    wall8 = wall.astype(f8)

    vr = np.asarray(v, dtype=np.float32).reshape(BH, T, D)
    vq = vr.astype(f8)                             # RNE quantize once
    # windows: frames 2..1025 (frames 1024,1025 zero) -> (BH, 16, 64, D)
    vpad = np.zeros((BH, NWIN * 64, D), f8)
    vpad[:, :T - 2] = vq[:, 2:, :]
    vw = vpad.reshape(BH, NWIN, 64, D)
    # xall[half, j, qall, win, bh] = vw[bh, win, j, 2*qall+half]
    xall = vw.transpose(3, 2, 1, 0).reshape(D // 2, 2, 64, NWIN, BH)
    xall = np.ascontiguousarray(xall.transpose(1, 2, 0, 3, 4))

    in_maps = []
    qc = PAIRS
    for cid in range(n_cores):
        sl = slice(cid * qc, (cid + 1) * qc)
        in_maps.append({
            "x": np.ascontiguousarray(xall[:, :, sl]).reshape(128, qc * FREE),
            "w": np.ascontiguousarray(wall8[:, :, sl]).reshape(128, qc * WCOL),
        })
    return in_maps


def kernel(v: np.ndarray, l_filter: np.ndarray, r_filter: np.ndarray,
           **_unused) -> np.ndarray:
    nc = _get_nc(PAIRS)
    in_maps = _make_in_maps(v, l_filter, r_filter)
    res = bass_utils.run_bass_kernel_spmd(nc, in_maps,
                                          core_ids=list(range(N_CORES)))
    vr = np.asarray(v, dtype=np.float32).reshape(BH, T, D)
    c = np.empty((BH, T, D), np.float32)
    qc = PAIRS
    for cid in range(N_CORES):
        yc = np.asarray(res.results[cid]["y"]).astype(np.float32)
        # [part (half, i), pair, win, bh] -> (bh, win, i, pair, half)
        yc = yc.reshape(2, 64, qc, NWIN, BH)
        yc[:, :, 1::2] = yc[::-1, :, 1::2]     # odd pairs: swapped PSUM rows
        yc = yc.transpose(4, 3, 1, 2, 0)
        c[:, :, 2 * qc * cid:2 * qc * (cid + 1)] = (
            yc.reshape(BH, T, 2 * qc) * (1.0 / WSCALE))

    # block-0 boundary: the dropped window -1 holds v[0], v[1] at rows 62, 63
    _, Wprev = _build_filters(l_filter, r_filter)
    corr0 = (np.einsum("id,nd->nid", Wprev[62], vr[:, 0, :].astype(np.float64))
             + np.einsum("id,nd->nid", Wprev[63], vr[:, 1, :].astype(np.float64)))
    c[:, 0:64, :] += corr0.astype(np.float32)
    out = vr + c
    return out.reshape(B, H, T, D)
